# revision 55
# baseline (speedup 1.0000x reference)
"""MultiHeadAttention Trainium2 kernel (8-core SPMD).

Reference computes, per batch b:
  q = (xq @ wq + bq) -> [S, H, D];  k, v likewise
  score[h] = q_h @ k_h^T;  masked with -1e9 where mask==0 BEFORE /sqrt(D)
  attn = softmax(score / 8)
  out = (attn @ v) @ wo + bo

Sharding: 8 cores = (batch b in 0..3) x (query half qh in 0..1).
Each core: 1024 queries x all 8 heads x full 2048 keys of its batch.
Outputs concatenate on host (no cross-core reduce needed).

Device dataflow (per core), k-major score layout:
  xqT/xkT/xvT are host-pre-transposed [E, S*] so projections need no
  on-device transposes:
    QT[hd, q]  = wq^T @ xqT   (lhsT=wq, rhs=xqT)   + bq (per-partition)
    KT[hd, k]  = wk^T @ xkT                        + bk
    V[k, hd]   = xvT^T @ wv   (lhsT=xvT, rhs=wv)   + bv (free-dim bcast)
  per head h:
    scoreT[k, q] = KT_h^T... lhsT=KT_h[64,kc], rhs=QT_h[64,q]  (two heads
      packed in the PE array concurrently via tile_position rows 0/64)
    p = exp(scoreT/8)           ACT, PSUM->SBUF bf16
    pm = p * maskT              DVE, bf16 (maskT host-transposed, 0/1)
    AOT[65, q] += V_aug_h^T... lhsT=V_aug[kc,65] (65th col = ones -> row 64
      of AOT accumulates the softmax denominator), rhs=pm[kc, q]
    AOT[0:64] *= 1/AOT[64]      reciprocal(DVE) + partition-bcast(Pool) + DVE
  out[q, e] = AOT^T... lhsT=AOT[hd, qc], rhs=wo; bo folded in as an extra
    ones-row contraction matmul; PSUM -> SBUF copy on ACT -> DMA out.

Scheduling (tuned against TimelineSim; ~200us model vs 246us for the
naive order):
  - DMA issue order = DMA_ENGINES service order (the model serializes all
    DMAs at ~360 GB/s): wv, xvT half 1, wq, xqT, wk, xkT halves, mask
    chunk 0, xvT half 2, mask 1-3, wo. Compute starts after ~1.5 MB.
  - V projection: sc 0-7 before the hp loop, sc 8-15 interleaved into
    hp0's kc loop (even kc); next head-pair's Q/K projection groups
    interleaved at odd kc (hp0) / kc 2..7 (hp1+).
  - Q/K bias adds run on ACT (Identity + per-partition bias) - GPSIMD
    cannot read PSUM on real HW; V add on DVE.
  - attnV accumulation emitted 3 kc behind its scores/exp/mask chain so
    the PE never stalls on the ACT->DVE pm latency.
  - Out-projection interleaved per q-tile into hp3's normalize, in
    qc-pairs ([P, 2, E] PSUM, one 1024-wide ACT copy + one DMA each).

Numerics: no max-subtraction needed (scores are O(1): inputs ~N(0,1),
weights*0.02 -> score std ~1.6, /8 -> exp args tiny). Masked entries are
exactly zero via the multiply. bf16 only on the S x S-sized tensors with
fp32 PSUM accumulation everywhere. Measured rel err vs fp32 reference:
5.3e-4 (gate 2e-2).
"""

import sys

for _p in ("/opt/trn_rl_repo",):
    if _p not in sys.path:
        sys.path.insert(0, _p)

import numpy as np

import concourse.bass as bass
from concourse import bacc
import concourse.tile as tile
import concourse.mybir as mybir
from concourse.bass_utils import run_bass_kernel_spmd

B, S, E = 4, 2048, 512
H, D = 8, 64
HD = H * D  # 512
SQ = S // 2  # queries per core
P = 128
F32 = mybir.dt.float32
BF16 = mybir.dt.float16  # 16-bit tensors use fp16 (11-bit mantissa)
EXP = mybir.ActivationFunctionType.Exp
MUL = mybir.AluOpType.mult
ADD = mybir.AluOpType.add

N_CORES = 8
EC = E // P  # 4 contraction chunks for projections
HC = HD // P  # 4 hd chunks
KC = S // P  # 16 key chunks
QT2 = SQ // 512  # 2 q-tiles of 512


def build_nc(reps: int = 1) -> bass.Bass:
    nc = bacc.Bacc()

    # ---- DRAM I/O (per-core shards, prepared on host) ----
    xqT_d = nc.dram_tensor("xqT", [E, SQ], BF16, kind="ExternalInput")
    xkT_d = nc.dram_tensor("xkT", [E, S], BF16, kind="ExternalInput")
    xvT_d = nc.dram_tensor("xvT", [E, S], BF16, kind="ExternalInput")
    maskT_d = nc.dram_tensor("maskT", [S, SQ], BF16, kind="ExternalInput")
    wq_d = nc.dram_tensor("wq", [E, HD], BF16, kind="ExternalInput")
    wk_d = nc.dram_tensor("wk", [E, HD], BF16, kind="ExternalInput")
    wv_d = nc.dram_tensor("wv", [E, HD], BF16, kind="ExternalInput")
    wo_d = nc.dram_tensor("wo_bf", [HD, E], BF16, kind="ExternalInput")
    bq_d = nc.dram_tensor("bq_pp", [P, HC], F32, kind="ExternalInput")
    bk_d = nc.dram_tensor("bk_pp", [P, HC], F32, kind="ExternalInput")
    bv_d = nc.dram_tensor("bv_bc", [P, HD], F32, kind="ExternalInput")
    bo_d = nc.dram_tensor("bo_row", [1, E], BF16, kind="ExternalInput")
    out_d = nc.dram_tensor("out", [SQ, E], F32, kind="ExternalOutput")

    with tile.TileContext(nc) as tc:
      for rep in range(reps):
        with (
            tc.tile_pool(name=f"singles{rep}", bufs=1) as singles,
            tc.tile_pool(name=f"work{rep}", bufs=3) as work,
            tc.tile_pool(name=f"pm{rep}", bufs=5) as pm_pool,
            tc.tile_pool(name=f"inputs{rep}", bufs=2) as inputs,
            # proj + scores share 2-bank slots: 4 banks; ao: 4 banks. 8 total.
            tc.tile_pool(name=f"psum_sc{rep}", bufs=2, space="PSUM") as psum_sc,
            tc.tile_pool(name=f"psum_ao{rep}", bufs=4, space="PSUM") as psum_ao,
        ):
            # ---- DMA order = DMA_ENGINES service order: V-path first (the
            # V projection is the first compute), then Q, K, mask, wo last.
            bq_sb = singles.tile([P, HC], F32, tag="bq")
            bk_sb = singles.tile([P, HC], F32, tag="bk")
            bv_sb = singles.tile([P, HD], F32, tag="bv")
            bo_sb = singles.tile([1, E], BF16, tag="bo")
            ones_sb = singles.tile([1, P], BF16, tag="ones1")
            nc.vector.memset(ones_sb[:], 1.0)
            for b_sb, b_d in ((bv_sb, bv_d), (bq_sb, bq_d), (bk_sb, bk_d), (bo_sb, bo_d)):
                nc.sync.dma_start(b_sb[:], b_d[:, :])
            QT_sb = singles.tile([P, HC, SQ], BF16, tag="QT")
            KT_sb = singles.tile([P, HC, S], BF16, tag="KT")
            V_sb = singles.tile([P, KC, H, 65], BF16, tag="V")
            nc.vector.memset(V_sb[:, :, :, 64:65], 1.0)

            maskT_sb = singles.tile([P, KC, SQ], BF16, tag="maskT")

            def dma_mask(mc):
                nc.sync.dma_start(
                    maskT_sb[:, 4 * mc : 4 * mc + 4, :],
                    maskT_d[512 * mc : 512 * (mc + 1), :].rearrange(
                        "(c p) q -> p c q", p=P
                    ),
                )

            wv_sb = inputs.tile([P, EC, HD], BF16, tag="wv", bufs=1)
            xvT_sb = inputs.tile([P, EC, S], BF16, tag="xv", bufs=1)
            nc.sync.dma_start(wv_sb[:], wv_d[:, :].rearrange("(c p) n -> p c n", p=P))

            def dma_xv(vh):
                vs = slice(vh * (S // 2), (vh + 1) * (S // 2))
                nc.sync.dma_start(
                    xvT_sb[:, :, vs], xvT_d[:, vs].rearrange("(c p) s -> p c s", p=P)
                )

            dma_xv(0)
            wq_sb = inputs.tile([P, EC, HD], BF16, tag="w", name="wq_sb")
            nc.sync.dma_start(wq_sb[:], wq_d[:, :].rearrange("(c p) n -> p c n", p=P))
            xqT_sb = inputs.tile([P, EC, SQ], BF16, tag="x", name="xqT_sb")
            nc.sync.dma_start(xqT_sb[:], xqT_d[:, :].rearrange("(c p) s -> p c s", p=P))
            wk_sb = inputs.tile([P, EC, HD], BF16, tag="w", name="wk_sb")
            nc.sync.dma_start(wk_sb[:], wk_d[:, :].rearrange("(c p) n -> p c n", p=P))
            xkT_sb = inputs.tile([P, EC, S], BF16, tag="x", name="xkT_sb")

            def dma_xk(kh):
                ks = slice(kh * (S // 2), (kh + 1) * (S // 2))
                nc.sync.dma_start(
                    xkT_sb[:, :, ks], xkT_d[:, ks].rearrange("(c p) s -> p c s", p=P)
                )

            dma_xk(0)
            dma_mask(0)
            dma_xk(1)
            dma_xv(1)
            dma_mask(1)
            dma_mask(2)
            dma_mask(3)
            wo_bf = singles.tile([P, HC, E], BF16, tag="wo_bf")
            nc.sync.dma_start(wo_bf[:], wo_d[:, :].rearrange("(c p) n -> p c n", p=P))

            AOT_sb = singles.tile([P, HC, SQ], BF16, tag="AOT")

            def vproj_group(sc):
                ps = psum_sc.tile([P, 512], F32, tag="scores", name=f"vps{sc}")
                for ec in range(EC):
                    nc.tensor.matmul(
                        ps[:],
                        lhsT=xvT_sb[:, ec, sc * P : (sc + 1) * P],
                        rhs=wv_sb[:, ec, :],
                        start=(ec == 0),
                        stop=(ec == EC - 1),
                    )
                with tc.high_priority(offset=40):
                    nc.vector.tensor_tensor(
                        V_sb[:, sc, :, 0:64],
                        ps[:].rearrange("p (h d) -> p h d", d=D),
                        bv_sb[:, :].rearrange("p (h d) -> p h d", d=D),
                        ADD,
                    )

            def proj_group(hc, dst, w_sb, x_sb, b_sb2, nt):
                ps = psum_sc.tile([P, 512], F32, tag="scores", name=f"pps{hc}{nt}")
                for ec in range(EC):
                    nc.tensor.matmul(
                        ps[:],
                        lhsT=w_sb[:, ec, hc * P : (hc + 1) * P],
                        rhs=x_sb[:, ec, nt * 512 : (nt + 1) * 512],
                        start=(ec == 0),
                        stop=(ec == EC - 1),
                    )
                with tc.high_priority(offset=40):
                    nc.scalar.activation(
                        dst[:, hc, nt * 512 : (nt + 1) * 512],
                        ps[:],
                        mybir.ActivationFunctionType.Identity,
                        bias=b_sb2[:, hc : hc + 1],
                    )

            def qkt_groups(hc):
                for dst, w_sb, x_sb, b_sb2, n_all in (
                    (QT_sb, wq_sb, xqT_sb, bq_sb, SQ),
                    (KT_sb, wk_sb, xkT_sb, bk_sb, S),
                ):
                    for nt in range(n_all // 512):
                        yield (hc, dst, w_sb, x_sb, b_sb2, nt)

            def project_qkt(hc):
                for g in qkt_groups(hc):
                    proj_group(*g)

            own_groups = {}
            for hp in range(HC):  # head pair = heads 2hp, 2hp+1
                if hp == 0:
                    for sc in range(KC // 2):
                        vproj_group(sc)
                    project_qkt(0)
                aos = []
                pmt = {}
                for ab in range(2):
                    ao = [
                        psum_ao.tile([65, 512], F32, tag="ao", name=f"ao_{ab}_{qt}")
                        for qt in range(QT2)
                    ]
                    aos.append(ao)
                ng = list(qkt_groups(hp + 1)) if hp + 1 < HC else []
                for kc in range(KC):
                    if hp == 0 and kc == 2:
                        vproj_group(8)
                        vproj_group(9)
                    elif hp == 0 and kc >= 4 and kc % 2 == 0:
                        vproj_group(9 + (kc - 2) // 2)  # kc=4..14 -> sc=10..15
                    if hp == 0:
                        if kc >= 3 and kc % 2 == 1 and (kc - 3) // 2 < len(ng):
                            proj_group(*ng[(kc - 3) // 2])
                    elif 2 <= kc < 2 + len(ng):
                        proj_group(*ng[kc - 2])
                    for ab in range(2):
                        h = 2 * hp + ab
                        pr0, pr1 = ab * 64, (ab + 1) * 64
                        if (ab, kc // 4) not in pmt:
                            pmt[(ab, kc // 4)] = pm_pool.tile(
                                [P, 4, SQ], BF16, tag="pm", name=f"pm_{ab}_{kc//4}"
                            )
                        pm = pmt[(ab, kc // 4)]
                        sc_ps = psum_sc.tile([P, SQ], F32, tag="scores")
                        for qt in range(QT2):
                            nc.tensor.matmul(
                                sc_ps[:, qt * 512 : (qt + 1) * 512],
                                lhsT=KT_sb[pr0:pr1, hp, kc * P : (kc + 1) * P],
                                rhs=QT_sb[pr0:pr1, hp, qt * 512 : (qt + 1) * 512],
                                start=True,
                                stop=True,
                                tile_position=(pr0, 0),
                            )
                        nc.scalar.activation(pm[:, kc % 4, :], sc_ps[:], EXP, scale=0.125)
                        nc.vector.tensor_tensor(
                            pm[:, kc % 4, :], pm[:, kc % 4, :], maskT_sb[:, kc, :], MUL
                        )
                    for ab in range(2):
                        for kcd in ([kc - 3] if kc > 2 else []) + ([kc - 2, kc - 1, kc] if kc == KC - 1 else []):
                            h = 2 * hp + ab
                            pmd = pmt[(ab, kcd // 4)]
                            for qt in range(QT2):
                                nc.tensor.matmul(
                                    aos[ab][qt][:],
                                    lhsT=V_sb[:, kcd, h, :],
                                    rhs=pmd[:, kcd % 4, qt * 512 : (qt + 1) * 512],
                                    start=(kcd == 0),
                                    stop=(kcd == KC - 1),
                                )
                # normalize: AOT[0:64] * (1/AOT[64]) bcast over partitions;
                # on the last head pair, out-projection follows per q-tile:
                # out[q, e] = AOT^T @ wo (+ bo via ones contraction row),
                # DMA'd straight from PSUM.
                for qt in range(QT2):
                    qs = slice(qt * 512, (qt + 1) * 512)
                    for ab in range(2):
                        pr0, pr1 = ab * 64, (ab + 1) * 64
                        rc = work.tile([1, 512], F32, tag="recip")
                        nc.vector.reciprocal(rc[:], aos[ab][qt][64:65, :])
                        rcb = work.tile([64, 512], F32, tag="rcb")
                        nc.gpsimd.partition_broadcast(rcb[:], rc[0:1, :])
                        nc.vector.tensor_tensor(
                            AOT_sb[pr0:pr1, hp, qs],
                            aos[ab][qt][0:64, :],
                            rcb[:],
                            MUL,
                        )
                    if hp == HC - 1:
                        for qp in range(2 * qt, 2 * (qt + 1)):
                            fps = psum_sc.tile(
                                [P, 2, E], F32, tag="scores", name=f"fout{qp}"
                            )
                            for half in range(2):
                                qc = 2 * qp + half
                                for hc in range(HC):
                                    nc.tensor.matmul(
                                        fps[:, half, :],
                                        lhsT=AOT_sb[:, hc, qc * P : (qc + 1) * P],
                                        rhs=wo_bf[:, hc, :],
                                        start=(hc == 0),
                                        stop=False,
                                    )
                                nc.tensor.matmul(
                                    fps[:, half, :],
                                    lhsT=ones_sb[:],
                                    rhs=bo_sb[:],
                                    start=False,
                                    stop=True,
                                )
                            fo = work.tile([P, 2, E], F32, tag="fout", bufs=2)
                            nc.scalar.copy(fo[:], fps[:])
                            nc.sync.dma_start(
                                out_d[2 * qp * P : 2 * (qp + 1) * P, :].rearrange(
                                    "(c p) n -> p c n", p=P
                                ),
                                fo[:],
                            )

    nc.finalize()
    return nc


_NC_CACHE = {}


def _get_nc(reps: int = 1):
    if reps not in _NC_CACHE:
        _NC_CACHE[reps] = build_nc(reps)
    return _NC_CACHE[reps]


def make_in_maps(input_q, input_k, input_v, mask, wq, bq, wk, bk, wv, bv, wo, bo):
    input_q = np.asarray(input_q, np.float32)
    input_k = np.asarray(input_k, np.float32)
    input_v = np.asarray(input_v, np.float32)
    mask = np.asarray(mask)
    f = np.float32
    h = np.float16
    wq, wk, wv = (np.ascontiguousarray(w).astype(h) for w in (wq, wk, wv))
    wo = np.ascontiguousarray(wo, f)
    bq_pp = np.ascontiguousarray(np.asarray(bq, f).reshape(HC, P).T)
    bk_pp = np.ascontiguousarray(np.asarray(bk, f).reshape(HC, P).T)
    bv_bc = np.ascontiguousarray(np.broadcast_to(np.asarray(bv, f), (P, HD)))
    bo_row = np.ascontiguousarray(np.asarray(bo, f).reshape(1, E)).astype(h)
    kT = [np.ascontiguousarray(input_k[b].T).astype(h) for b in range(B)]
    vT = [np.ascontiguousarray(input_v[b].T).astype(h) for b in range(B)]
    in_maps = []
    for c in range(N_CORES):
        b, qh = c // 2, c % 2
        qs = slice(qh * SQ, (qh + 1) * SQ)
        in_maps.append(
            {
                "xqT": np.ascontiguousarray(input_q[b, qs].T).astype(h),
                "xkT": kT[b],
                "xvT": vT[b],
                "maskT": np.ascontiguousarray(mask[b, qs].T).astype(np.float16),
                "wq": wq,
                "wk": wk,
                "wv": wv,
                "wo_bf": wo.astype(np.float16),
                "bq_pp": bq_pp,
                "bk_pp": bk_pp,
                "bv_bc": bv_bc,
                "bo_row": bo_row,
            }
        )
    return in_maps


def kernel(input_q, input_k, input_v, mask, wq, bq, wk, bk, wv, bv, wo, bo, **_kw):
    nc = _get_nc()
    in_maps = make_in_maps(
        input_q, input_k, input_v, mask, wq, bq, wk, bk, wv, bv, wo, bo
    )
    res = run_bass_kernel_spmd(nc, in_maps, core_ids=list(range(N_CORES)))
    out = np.empty((B, S, E), np.float32)
    for c in range(N_CORES):
        b, qh = c // 2, c % 2
        out[b, qh * SQ : (qh + 1) * SQ] = res.results[c]["out"]
    return out


if __name__ == "__main__":
    rng = np.random.default_rng(0)
    print("building...")
    _get_nc()
    print("built ok")



# revision 72
# speedup vs baseline: 1.0167x; 1.0167x over previous
"""MultiHeadAttention Trainium2 kernel (8-core SPMD).

Reference computes, per batch b:
  q = (xq @ wq + bq) -> [S, H, D];  k, v likewise
  score[h] = q_h @ k_h^T;  masked with -1e9 where mask==0 BEFORE /sqrt(D)
  attn = softmax(score / 8)
  out = (attn @ v) @ wo + bo

Sharding: 8 cores = (batch b in 0..3) x (query half qh in 0..1).
Each core: 1024 queries x all 8 heads x full 2048 keys of its batch.
Outputs concatenate on host (no cross-core reduce needed).

Device dataflow (per core), k-major score layout:
  xqT/xkT/xvT are host-pre-transposed [E, S*] so projections need no
  on-device transposes:
    QT[hd, q]  = wq^T @ xqT   (lhsT=wq, rhs=xqT)   + bq (per-partition)
    KT[hd, k]  = wk^T @ xkT                        + bk
    V[k, hd]   = xvT^T @ wv   (lhsT=xvT, rhs=wv)   + bv (free-dim bcast)
  per head h:
    scoreT[k, q] = KT_h^T... lhsT=KT_h[64,kc], rhs=QT_h[64,q]  (two heads
      packed in the PE array concurrently via tile_position rows 0/64)
    p = exp(scoreT/8)           ACT, PSUM->SBUF bf16
    pm = p * maskT              DVE, bf16 (maskT host-transposed, 0/1)
    AOT[65, q] += V_aug_h^T... lhsT=V_aug[kc,65] (65th col = ones -> row 64
      of AOT accumulates the softmax denominator), rhs=pm[kc, q]
    AOT[0:64] *= 1/AOT[64]      reciprocal(DVE) + partition-bcast(Pool) + DVE
  out[q, e] = AOT^T... lhsT=AOT[hd, qc], rhs=wo; bo folded in as an extra
    ones-row contraction matmul; PSUM -> SBUF copy on ACT -> DMA out.

Scheduling (tuned against TimelineSim; ~200us model vs 246us for the
naive order):
  - DMA issue order = DMA_ENGINES service order (the model serializes all
    DMAs at ~360 GB/s): wv, xvT half 1, wq, xqT, wk, xkT halves, mask
    chunk 0, xvT half 2, mask 1-3, wo. Compute starts after ~1.5 MB.
  - V projection: sc 0-7 before the hp loop, sc 8-15 interleaved into
    hp0's kc loop (even kc); next head-pair's Q/K projection groups
    interleaved at odd kc (hp0) / kc 2..7 (hp1+).
  - Q/K bias adds run on ACT (Identity + per-partition bias) - GPSIMD
    cannot read PSUM on real HW; V add on DVE.
  - attnV accumulation emitted 3 kc behind its scores/exp/mask chain so
    the PE never stalls on the ACT->DVE pm latency.
  - Out-projection interleaved per q-tile into hp3's normalize, in
    qc-pairs ([P, 2, E] PSUM, one 1024-wide ACT copy + one DMA each).

Numerics: no max-subtraction needed (scores are O(1): inputs ~N(0,1),
weights*0.02 -> score std ~1.6, /8 -> exp args tiny). Masked entries are
exactly zero via the multiply. bf16 only on the S x S-sized tensors with
fp32 PSUM accumulation everywhere. Measured rel err vs fp32 reference:
5.3e-4 (gate 2e-2).
"""

import sys

for _p in ("/opt/trn_rl_repo",):
    if _p not in sys.path:
        sys.path.insert(0, _p)

import numpy as np

import concourse.bass as bass
from concourse import bacc
import concourse.tile as tile
import concourse.mybir as mybir
from concourse.bass_utils import run_bass_kernel_spmd

B, S, E = 4, 2048, 512
H, D = 8, 64
HD = H * D  # 512
SQ = S // 2  # queries per core
P = 128
F32 = mybir.dt.float32
BF16 = mybir.dt.float16  # 16-bit tensors use fp16 (11-bit mantissa)
EXP = mybir.ActivationFunctionType.Exp
MUL = mybir.AluOpType.mult
ADD = mybir.AluOpType.add

N_CORES = 8
EC = E // P  # 4 contraction chunks for projections
HC = HD // P  # 4 hd chunks
KC = S // P  # 16 key chunks
QT2 = SQ // 512  # 2 q-tiles of 512


def build_nc(reps: int = 1) -> bass.Bass:
    nc = bacc.Bacc()

    # ---- DRAM I/O (per-core shards, prepared on host) ----
    xqT_d = nc.dram_tensor("xqT", [E, SQ], BF16, kind="ExternalInput")
    xkT_d = nc.dram_tensor("xkT", [E, S], BF16, kind="ExternalInput")
    xvT_d = nc.dram_tensor("xvT", [E, S], BF16, kind="ExternalInput")
    maskT_d = nc.dram_tensor("maskT", [S, SQ], BF16, kind="ExternalInput")
    wq_d = nc.dram_tensor("wq", [E, HD], BF16, kind="ExternalInput")
    wk_d = nc.dram_tensor("wk", [E, HD], BF16, kind="ExternalInput")
    wv_d = nc.dram_tensor("wv", [E, HD], BF16, kind="ExternalInput")
    wo_d = nc.dram_tensor("wo_bf", [HD, E], BF16, kind="ExternalInput")
    bq_d = nc.dram_tensor("bq_pp", [P, HC], F32, kind="ExternalInput")
    bk_d = nc.dram_tensor("bk_pp", [P, HC], F32, kind="ExternalInput")
    bv_d = nc.dram_tensor("bv_bc", [P, HD], F32, kind="ExternalInput")
    bo_d = nc.dram_tensor("bo_row", [1, E], BF16, kind="ExternalInput")
    out_d = nc.dram_tensor("out", [SQ, E], F32, kind="ExternalOutput")

    with tile.TileContext(nc) as tc:
      for rep in range(reps):
        with (
            tc.tile_pool(name=f"singles{rep}", bufs=1) as singles,
            tc.tile_pool(name=f"work{rep}", bufs=3) as work,
            tc.tile_pool(name=f"pm{rep}", bufs=5) as pm_pool,
            tc.tile_pool(name=f"inputs{rep}", bufs=2) as inputs,
            # proj + scores share 2-bank slots: 4 banks; ao: 4 banks. 8 total.
            tc.tile_pool(name=f"psum_sc{rep}", bufs=2, space="PSUM") as psum_sc,
            tc.tile_pool(name=f"psum_ao{rep}", bufs=4, space="PSUM") as psum_ao,
        ):
            # ---- DMA order = DMA_ENGINES service order: V-path first (the
            # V projection is the first compute), then Q, K, mask, wo last.
            bq_sb = singles.tile([P, HC], F32, tag="bq")
            bk_sb = singles.tile([P, HC], F32, tag="bk")
            bv_sb = singles.tile([P, HD], F32, tag="bv")
            bo_sb = singles.tile([1, E], BF16, tag="bo")
            ones_sb = singles.tile([1, P], BF16, tag="ones1")
            nc.vector.memset(ones_sb[:], 1.0)

            QT_sb = singles.tile([P, HC, SQ], BF16, tag="QT")
            KT_sb = singles.tile([P, HC, S], BF16, tag="KT")
            V_sb = singles.tile([P, KC, H, 65], BF16, tag="V")
            nc.vector.memset(V_sb[:, :, :, 64:65], 1.0)

            maskT_sb = singles.tile([P, KC, SQ], BF16, tag="maskT")

            def dma_mask(mc):
                nc.sync.dma_start(
                    maskT_sb[:, 4 * mc : 4 * mc + 4, :],
                    maskT_d[512 * mc : 512 * (mc + 1), :].rearrange(
                        "(c p) q -> p c q", p=P
                    ),
                )

            wv_sb = inputs.tile([P, EC, HD], BF16, tag="wv", bufs=1)
            xvT_sb = inputs.tile([P, EC, S], BF16, tag="xv", bufs=1)
            nc.sync.dma_start(wv_sb[:], wv_d[:, :].rearrange("(c p) n -> p c n", p=P))

            def dma_xv(vh):
                vs = slice(vh * (S // 2), (vh + 1) * (S // 2))
                nc.sync.dma_start(
                    xvT_sb[:, :, vs], xvT_d[:, vs].rearrange("(c p) s -> p c s", p=P)
                )

            dma_xv(0)
            nc.sync.dma_start(bv_sb[:], bv_d[:, :])
            nc.sync.dma_start(bq_sb[:], bq_d[:, :])
            nc.sync.dma_start(bk_sb[:], bk_d[:, :])
            wq_sb = inputs.tile([P, EC, HD], BF16, tag="w", name="wq_sb")
            nc.sync.dma_start(wq_sb[:], wq_d[:, :].rearrange("(c p) n -> p c n", p=P))
            xqT_sb = inputs.tile([P, EC, SQ], BF16, tag="x", name="xqT_sb")
            nc.sync.dma_start(xqT_sb[:], xqT_d[:, :].rearrange("(c p) s -> p c s", p=P))
            wk_sb = inputs.tile([P, EC, HD], BF16, tag="w", name="wk_sb")
            nc.sync.dma_start(wk_sb[:], wk_d[:, :].rearrange("(c p) n -> p c n", p=P))
            xkT_sb = inputs.tile([P, EC, S], BF16, tag="x", name="xkT_sb")

            def dma_xk(kh):
                ks = slice(kh * (S // 2), (kh + 1) * (S // 2))
                nc.sync.dma_start(
                    xkT_sb[:, :, ks], xkT_d[:, ks].rearrange("(c p) s -> p c s", p=P)
                )

            dma_xk(0)
            dma_mask(0)
            dma_xk(1)
            dma_xv(1)
            dma_mask(1)
            dma_mask(2)
            dma_mask(3)
            wo_bf = singles.tile([P, HC, E], BF16, tag="wo_bf")
            nc.sync.dma_start(wo_bf[:], wo_d[:, :].rearrange("(c p) n -> p c n", p=P))
            nc.sync.dma_start(bo_sb[:], bo_d[:, :])

            AOT_sb = singles.tile([P, HC, SQ], BF16, tag="AOT")

            def vproj_group(sc):
                ps = psum_sc.tile([P, 512], F32, tag="scores", name=f"vps{sc}")
                for ec in range(EC):
                    nc.tensor.matmul(
                        ps[:],
                        lhsT=xvT_sb[:, ec, sc * P : (sc + 1) * P],
                        rhs=wv_sb[:, ec, :],
                        start=(ec == 0),
                        stop=(ec == EC - 1),
                    )
                with tc.high_priority(offset=40):
                    nc.vector.tensor_tensor(
                        V_sb[:, sc, :, 0:64],
                        ps[:].rearrange("p (h d) -> p h d", d=D),
                        bv_sb[:, :].rearrange("p (h d) -> p h d", d=D),
                        ADD,
                    )

            def proj_group(hc, dst, w_sb, x_sb, b_sb2, nt):
                ps = psum_sc.tile([P, 512], F32, tag="scores", name=f"pps{hc}{nt}")
                for ec in range(EC):
                    nc.tensor.matmul(
                        ps[:],
                        lhsT=w_sb[:, ec, hc * P : (hc + 1) * P],
                        rhs=x_sb[:, ec, nt * 512 : (nt + 1) * 512],
                        start=(ec == 0),
                        stop=(ec == EC - 1),
                    )
                with tc.high_priority(offset=40):
                    nc.scalar.activation(
                        dst[:, hc, nt * 512 : (nt + 1) * 512],
                        ps[:],
                        mybir.ActivationFunctionType.Identity,
                        bias=b_sb2[:, hc : hc + 1],
                    )

            def qkt_groups(hc):
                for dst, w_sb, x_sb, b_sb2, n_all in (
                    (QT_sb, wq_sb, xqT_sb, bq_sb, SQ),
                    (KT_sb, wk_sb, xkT_sb, bk_sb, S),
                ):
                    for nt in range(n_all // 512):
                        yield (hc, dst, w_sb, x_sb, b_sb2, nt)

            def project_qkt(hc):
                for g in qkt_groups(hc):
                    proj_group(*g)

            own_groups = {}
            for hp in range(HC):  # head pair = heads 2hp, 2hp+1
                if hp == 0:
                    for sc in range(KC // 2):
                        vproj_group(sc)
                    project_qkt(0)
                aos = []
                pmt = {}
                for ab in range(2):
                    ao = [
                        psum_ao.tile([65, 512], F32, tag="ao", name=f"ao_{ab}_{qt}")
                        for qt in range(QT2)
                    ]
                    aos.append(ao)
                ng = list(qkt_groups(hp + 1)) if hp + 1 < HC else []
                for kc in range(KC):
                    if hp == 0 and kc == 2:
                        vproj_group(8)
                        vproj_group(9)
                    elif hp == 0 and kc >= 4 and kc % 2 == 0:
                        vproj_group(9 + (kc - 2) // 2)  # kc=4..14 -> sc=10..15
                    if hp == 0:
                        if kc >= 3 and kc % 2 == 1 and (kc - 3) // 2 < len(ng):
                            proj_group(*ng[(kc - 3) // 2])
                    elif 2 <= kc < 2 + len(ng):
                        proj_group(*ng[kc - 2])
                    for ab in range(2):
                        h = 2 * hp + ab
                        pr0, pr1 = ab * 64, (ab + 1) * 64
                        if (ab, kc // 4) not in pmt:
                            pmt[(ab, kc // 4)] = pm_pool.tile(
                                [P, 4, SQ], BF16, tag="pm", name=f"pm_{ab}_{kc//4}"
                            )
                        pm = pmt[(ab, kc // 4)]
                        sc_ps = psum_sc.tile([P, SQ], F32, tag="scores")
                        for qt in range(QT2):
                            nc.tensor.matmul(
                                sc_ps[:, qt * 512 : (qt + 1) * 512],
                                lhsT=KT_sb[pr0:pr1, hp, kc * P : (kc + 1) * P],
                                rhs=QT_sb[pr0:pr1, hp, qt * 512 : (qt + 1) * 512],
                                start=True,
                                stop=True,
                                tile_position=(pr0, 0),
                            )
                        nc.scalar.activation(pm[:, kc % 4, :], sc_ps[:], EXP, scale=0.125)
                        nc.vector.tensor_tensor(
                            pm[:, kc % 4, :], pm[:, kc % 4, :], maskT_sb[:, kc, :], MUL
                        )
                    for ab in range(2):
                        sh = 3
                        for kcd in ([kc - sh] if kc >= sh else []) + (list(range(kc - sh + 1, kc + 1)) if kc == KC - 1 else []):
                            h = 2 * hp + ab
                            pmd = pmt[(ab, kcd // 4)]
                            for qt in range(QT2):
                                nc.tensor.matmul(
                                    aos[ab][qt][:],
                                    lhsT=V_sb[:, kcd, h, :],
                                    rhs=pmd[:, kcd % 4, qt * 512 : (qt + 1) * 512],
                                    start=(kcd == 0),
                                    stop=(kcd == KC - 1),
                                )
                # normalize: AOT[0:64] * (1/AOT[64]) bcast over partitions;
                # on the last head pair, out-projection per q-tile:
                # hc 0..2 partials start as soon as PSUM slots free (AOT for
                # hp0-2 is long done); only hc3 + bias wait on the normalize.
                fouts = {}

                def fout_partial(qp):
                    fps = psum_sc.tile([P, 2, E], F32, tag="scores", name=f"fout{qp}")
                    fouts[qp] = fps
                    for half in range(2):
                        qc = 2 * qp + half
                        for hc in range(HC - 1):
                            nc.tensor.matmul(
                                fps[:, half, :],
                                lhsT=AOT_sb[:, hc, qc * P : (qc + 1) * P],
                                rhs=wo_bf[:, hc, :],
                                start=(hc == 0),
                                stop=False,
                            )

                def fout_finish(qp):
                    fps = fouts[qp]
                    for half in range(2):
                        qc = 2 * qp + half
                        nc.tensor.matmul(
                            fps[:, half, :],
                            lhsT=AOT_sb[:, HC - 1, qc * P : (qc + 1) * P],
                            rhs=wo_bf[:, HC - 1, :],
                            start=False,
                            stop=False,
                        )
                        nc.tensor.matmul(
                            fps[:, half, :],
                            lhsT=ones_sb[:],
                            rhs=bo_sb[:],
                            start=False,
                            stop=True,
                        )
                    fo = work.tile([P, 2, E], F32, tag="fout", bufs=2)
                    nc.scalar.copy(fo[:], fps[:])
                    nc.sync.dma_start(
                        out_d[2 * qp * P : 2 * (qp + 1) * P, :].rearrange(
                            "(c p) n -> p c n", p=P
                        ),
                        fo[:],
                    )

                if hp == HC - 1:
                    fout_partial(0)
                    fout_partial(1)
                for qt in range(QT2):
                    qs = slice(qt * 512, (qt + 1) * 512)
                    for ab in range(2):
                        pr0, pr1 = ab * 64, (ab + 1) * 64
                        with tc.high_priority(offset=200):
                            rc = work.tile([1, 512], F32, tag="recip")
                            nc.vector.reciprocal(rc[:], aos[ab][qt][64:65, :])
                            rcb = work.tile([64, 512], F32, tag="rcb")
                            nc.gpsimd.partition_broadcast(rcb[:], rc[0:1, :])
                            nc.vector.tensor_tensor(
                                AOT_sb[pr0:pr1, hp, qs],
                                aos[ab][qt][0:64, :],
                                rcb[:],
                                MUL,
                            )
                    if hp == HC - 1:
                        for qp in range(2 * qt, 2 * (qt + 1)):
                            fout_finish(qp)
                        if qt == 0:
                            fout_partial(2)
                            fout_partial(3)

    nc.finalize()
    return nc


_NC_CACHE = {}


def _get_nc(reps: int = 1):
    if reps not in _NC_CACHE:
        _NC_CACHE[reps] = build_nc(reps)
    return _NC_CACHE[reps]


def make_in_maps(input_q, input_k, input_v, mask, wq, bq, wk, bk, wv, bv, wo, bo):
    input_q = np.asarray(input_q, np.float32)
    input_k = np.asarray(input_k, np.float32)
    input_v = np.asarray(input_v, np.float32)
    mask = np.asarray(mask)
    f = np.float32
    h = np.float16
    wq, wk, wv = (np.ascontiguousarray(w).astype(h) for w in (wq, wk, wv))
    wo = np.ascontiguousarray(wo, f)
    bq_pp = np.ascontiguousarray(np.asarray(bq, f).reshape(HC, P).T)
    bk_pp = np.ascontiguousarray(np.asarray(bk, f).reshape(HC, P).T)
    bv_bc = np.ascontiguousarray(np.broadcast_to(np.asarray(bv, f), (P, HD)))
    bo_row = np.ascontiguousarray(np.asarray(bo, f).reshape(1, E)).astype(h)
    kT = [np.ascontiguousarray(input_k[b].T).astype(h) for b in range(B)]
    vT = [np.ascontiguousarray(input_v[b].T).astype(h) for b in range(B)]
    in_maps = []
    for c in range(N_CORES):
        b, qh = c // 2, c % 2
        qs = slice(qh * SQ, (qh + 1) * SQ)
        in_maps.append(
            {
                "xqT": np.ascontiguousarray(input_q[b, qs].T).astype(h),
                "xkT": kT[b],
                "xvT": vT[b],
                "maskT": np.ascontiguousarray(mask[b, qs].T).astype(np.float16),
                "wq": wq,
                "wk": wk,
                "wv": wv,
                "wo_bf": wo.astype(np.float16),
                "bq_pp": bq_pp,
                "bk_pp": bk_pp,
                "bv_bc": bv_bc,
                "bo_row": bo_row,
            }
        )
    return in_maps


def kernel(input_q, input_k, input_v, mask, wq, bq, wk, bk, wv, bv, wo, bo, **_kw):
    nc = _get_nc()
    in_maps = make_in_maps(
        input_q, input_k, input_v, mask, wq, bq, wk, bk, wv, bv, wo, bo
    )
    res = run_bass_kernel_spmd(nc, in_maps, core_ids=list(range(N_CORES)))
    out = np.empty((B, S, E), np.float32)
    for c in range(N_CORES):
        b, qh = c // 2, c % 2
        out[b, qh * SQ : (qh + 1) * SQ] = res.results[c]["out"]
    return out


if __name__ == "__main__":
    rng = np.random.default_rng(0)
    print("building...")
    _get_nc()
    print("built ok")



# revision 75
# speedup vs baseline: 1.0220x; 1.0053x over previous
"""MultiHeadAttention Trainium2 kernel (8-core SPMD).

Reference computes, per batch b:
  q = (xq @ wq + bq) -> [S, H, D];  k, v likewise
  score[h] = q_h @ k_h^T;  masked with -1e9 where mask==0 BEFORE /sqrt(D)
  attn = softmax(score / 8)
  out = (attn @ v) @ wo + bo

Sharding: 8 cores = (batch b in 0..3) x (query half qh in 0..1).
Each core: 1024 queries x all 8 heads x full 2048 keys of its batch.
Outputs concatenate on host (no cross-core reduce needed).

Device dataflow (per core), k-major score layout:
  xqT/xkT/xvT are host-pre-transposed [E, S*] so projections need no
  on-device transposes:
    QT[hd, q]  = wq^T @ xqT   (lhsT=wq, rhs=xqT)   + bq (per-partition)
    KT[hd, k]  = wk^T @ xkT                        + bk
    V[k, hd]   = xvT^T @ wv   (lhsT=xvT, rhs=wv)   + bv (free-dim bcast)
  per head h:
    scoreT[k, q] = KT_h^T... lhsT=KT_h[64,kc], rhs=QT_h[64,q]  (two heads
      packed in the PE array concurrently via tile_position rows 0/64)
    p = exp(scoreT/8)           ACT, PSUM->SBUF bf16
    pm = p * maskT              DVE, bf16 (maskT host-transposed, 0/1)
    AOT[65, q] += V_aug_h^T... lhsT=V_aug[kc,65] (65th col = ones -> row 64
      of AOT accumulates the softmax denominator), rhs=pm[kc, q]
    AOT[0:64] *= 1/AOT[64]      reciprocal(DVE) + partition-bcast(Pool) + DVE
  out[q, e] = AOT^T... lhsT=AOT[hd, qc], rhs=wo; bo folded in as an extra
    ones-row contraction matmul; PSUM -> SBUF copy on ACT -> DMA out.

Scheduling (tuned against TimelineSim; ~200us model vs 246us for the
naive order):
  - DMA issue order = DMA_ENGINES service order (the model serializes all
    DMAs at ~360 GB/s): wv, xvT half 1, wq, xqT, wk, xkT halves, mask
    chunk 0, xvT half 2, mask 1-3, wo. Compute starts after ~1.5 MB.
  - V projection: sc 0-7 before the hp loop, sc 8-15 interleaved into
    hp0's kc loop (even kc); next head-pair's Q/K projection groups
    interleaved at odd kc (hp0) / kc 2..7 (hp1+).
  - Q/K bias adds run on ACT (Identity + per-partition bias) - GPSIMD
    cannot read PSUM on real HW; V add on DVE.
  - attnV accumulation emitted 3 kc behind its scores/exp/mask chain so
    the PE never stalls on the ACT->DVE pm latency.
  - Out-projection interleaved per q-tile into hp3's normalize, in
    qc-pairs ([P, 2, E] PSUM, one 1024-wide ACT copy + one DMA each).

Numerics: no max-subtraction needed (scores are O(1): inputs ~N(0,1),
weights*0.02 -> score std ~1.6, /8 -> exp args tiny). Masked entries are
exactly zero via the multiply. bf16 only on the S x S-sized tensors with
fp32 PSUM accumulation everywhere. Measured rel err vs fp32 reference:
5.3e-4 (gate 2e-2).
"""

import sys

for _p in ("/opt/trn_rl_repo",):
    if _p not in sys.path:
        sys.path.insert(0, _p)

import numpy as np

import concourse.bass as bass
from concourse import bacc
import concourse.tile as tile
import concourse.mybir as mybir
from concourse.bass_utils import run_bass_kernel_spmd

B, S, E = 4, 2048, 512
H, D = 8, 64
HD = H * D  # 512
SQ = S // 2  # queries per core
P = 128
F32 = mybir.dt.float32
BF16 = mybir.dt.float16  # 16-bit tensors use fp16 (11-bit mantissa)
EXP = mybir.ActivationFunctionType.Exp
MUL = mybir.AluOpType.mult
ADD = mybir.AluOpType.add

N_CORES = 8
EC = E // P  # 4 contraction chunks for projections
HC = HD // P  # 4 hd chunks
KC = S // P  # 16 key chunks
QT2 = SQ // 512  # 2 q-tiles of 512


def build_nc(reps: int = 1) -> bass.Bass:
    nc = bacc.Bacc()

    # ---- DRAM I/O (per-core shards, prepared on host) ----
    xqT_d = nc.dram_tensor("xqT", [E, SQ], BF16, kind="ExternalInput")
    xkT_d = nc.dram_tensor("xkT", [E, S], BF16, kind="ExternalInput")
    xvT_d = nc.dram_tensor("xvT", [E, S], BF16, kind="ExternalInput")
    maskT_d = nc.dram_tensor("maskT", [S, SQ], BF16, kind="ExternalInput")
    wq_d = nc.dram_tensor("wq", [E, HD], BF16, kind="ExternalInput")
    wk_d = nc.dram_tensor("wk", [E, HD], BF16, kind="ExternalInput")
    wv_d = nc.dram_tensor("wv", [E, HD], BF16, kind="ExternalInput")
    wo_d = nc.dram_tensor("wo_bf", [HD, E], BF16, kind="ExternalInput")
    bq_d = nc.dram_tensor("bq_pp", [P, HC], F32, kind="ExternalInput")
    bk_d = nc.dram_tensor("bk_pp", [P, HC], F32, kind="ExternalInput")
    bv_d = nc.dram_tensor("bv_bc", [P, HD], F32, kind="ExternalInput")
    bo_d = nc.dram_tensor("bo_row", [1, E], BF16, kind="ExternalInput")
    out_d = nc.dram_tensor("out", [SQ, E], F32, kind="ExternalOutput")

    with tile.TileContext(nc) as tc:
      for rep in range(reps):
        with (
            tc.tile_pool(name=f"singles{rep}", bufs=1) as singles,
            tc.tile_pool(name=f"work{rep}", bufs=3) as work,
            tc.tile_pool(name=f"pm{rep}", bufs=5) as pm_pool,
            tc.tile_pool(name=f"inputs{rep}", bufs=2) as inputs,
            # proj + scores share 2-bank slots: 4 banks; ao: 4 banks. 8 total.
            tc.tile_pool(name=f"psum_sc{rep}", bufs=2, space="PSUM") as psum_sc,
            tc.tile_pool(name=f"psum_ao{rep}", bufs=4, space="PSUM") as psum_ao,
        ):
            # ---- DMA order = DMA_ENGINES service order: V-path first (the
            # V projection is the first compute), then Q, K, mask, wo last.
            bq_sb = singles.tile([P, HC], F32, tag="bq")
            bk_sb = singles.tile([P, HC], F32, tag="bk")
            bv_sb = singles.tile([P, HD], F32, tag="bv")
            bo_sb = singles.tile([1, E], BF16, tag="bo")
            ones_sb = singles.tile([1, P], BF16, tag="ones1")
            nc.vector.memset(ones_sb[:], 1.0)

            QT_sb = singles.tile([P, HC, SQ], BF16, tag="QT")
            KT_sb = singles.tile([P, HC, S], BF16, tag="KT")
            V_sb = singles.tile([P, KC, H, 65], BF16, tag="V")
            nc.vector.memset(V_sb[:, :, :, 64:65], 1.0)

            maskT_sb = singles.tile([P, KC, SQ], BF16, tag="maskT")

            def dma_mask(mc):
                nc.sync.dma_start(
                    maskT_sb[:, 4 * mc : 4 * mc + 4, :],
                    maskT_d[512 * mc : 512 * (mc + 1), :].rearrange(
                        "(c p) q -> p c q", p=P
                    ),
                )

            wv_sb = inputs.tile([P, EC, HD], BF16, tag="wv", bufs=1)
            xvT_sb = inputs.tile([P, EC, S], BF16, tag="xv", bufs=1)
            nc.sync.dma_start(wv_sb[:], wv_d[:, :].rearrange("(c p) n -> p c n", p=P))

            def dma_xv(vh):
                vs = slice(vh * (S // 2), (vh + 1) * (S // 2))
                nc.sync.dma_start(
                    xvT_sb[:, :, vs], xvT_d[:, vs].rearrange("(c p) s -> p c s", p=P)
                )

            dma_xv(0)
            nc.sync.dma_start(bv_sb[:], bv_d[:, :])
            nc.sync.dma_start(bq_sb[:], bq_d[:, :])
            nc.sync.dma_start(bk_sb[:], bk_d[:, :])
            wq_sb = inputs.tile([P, EC, HD], BF16, tag="w", name="wq_sb")
            nc.sync.dma_start(wq_sb[:], wq_d[:, :].rearrange("(c p) n -> p c n", p=P))
            xqT_sb = inputs.tile([P, EC, SQ], BF16, tag="x", name="xqT_sb")
            nc.sync.dma_start(xqT_sb[:], xqT_d[:, :].rearrange("(c p) s -> p c s", p=P))
            wk_sb = inputs.tile([P, EC, HD], BF16, tag="w", name="wk_sb")
            nc.sync.dma_start(wk_sb[:], wk_d[:, :].rearrange("(c p) n -> p c n", p=P))
            xkT_sb = inputs.tile([P, EC, S], BF16, tag="x", name="xkT_sb")

            def dma_xk(kh):
                ks = slice(kh * (S // 2), (kh + 1) * (S // 2))
                nc.sync.dma_start(
                    xkT_sb[:, :, ks], xkT_d[:, ks].rearrange("(c p) s -> p c s", p=P)
                )

            dma_xk(0)
            dma_mask(0)
            dma_xk(1)
            dma_xv(1)
            dma_mask(1)
            dma_mask(2)
            dma_mask(3)
            wo_bf = singles.tile([P, HC, E], BF16, tag="wo_bf")
            nc.sync.dma_start(wo_bf[:], wo_d[:, :].rearrange("(c p) n -> p c n", p=P))
            nc.sync.dma_start(bo_sb[:], bo_d[:, :])

            AOT_sb = singles.tile([P, HC, SQ], BF16, tag="AOT")

            def vproj_group(sc):
                ps = psum_sc.tile([P, 512], F32, tag="scores", name=f"vps{sc}")
                for ec in range(EC):
                    nc.tensor.matmul(
                        ps[:],
                        lhsT=xvT_sb[:, ec, sc * P : (sc + 1) * P],
                        rhs=wv_sb[:, ec, :],
                        start=(ec == 0),
                        stop=(ec == EC - 1),
                    )
                with tc.high_priority(offset=40):
                    nc.vector.tensor_tensor(
                        V_sb[:, sc, :, 0:64],
                        ps[:].rearrange("p (h d) -> p h d", d=D),
                        bv_sb[:, :].rearrange("p (h d) -> p h d", d=D),
                        ADD,
                    )

            def proj_group(hc, dst, w_sb, x_sb, b_sb2, nt):
                ps = psum_sc.tile([P, 512], F32, tag="scores", name=f"pps{hc}{nt}")
                for ec in range(EC):
                    nc.tensor.matmul(
                        ps[:],
                        lhsT=w_sb[:, ec, hc * P : (hc + 1) * P],
                        rhs=x_sb[:, ec, nt * 512 : (nt + 1) * 512],
                        start=(ec == 0),
                        stop=(ec == EC - 1),
                    )
                with tc.high_priority(offset=40):
                    nc.scalar.activation(
                        dst[:, hc, nt * 512 : (nt + 1) * 512],
                        ps[:],
                        mybir.ActivationFunctionType.Identity,
                        bias=b_sb2[:, hc : hc + 1],
                    )

            def qkt_groups(hc):
                for dst, w_sb, x_sb, b_sb2, n_all in (
                    (QT_sb, wq_sb, xqT_sb, bq_sb, SQ),
                    (KT_sb, wk_sb, xkT_sb, bk_sb, S),
                ):
                    for nt in range(n_all // 512):
                        yield (hc, dst, w_sb, x_sb, b_sb2, nt)

            def project_qkt(hc):
                for g in qkt_groups(hc):
                    proj_group(*g)

            own_groups = {}
            for hp in range(HC):  # head pair = heads 2hp, 2hp+1
                if hp == 0:
                    for sc in range(KC // 2):
                        vproj_group(sc)
                    project_qkt(0)
                aos = []
                pmt = {}
                for ab in range(2):
                    ao = [
                        psum_ao.tile([65, 512], F32, tag="ao", name=f"ao_{ab}_{qt}")
                        for qt in range(QT2)
                    ]
                    aos.append(ao)
                ng = list(qkt_groups(hp + 1)) if hp + 1 < HC else []
                for kc in range(KC):
                    if hp == 0 and kc == 2:
                        vproj_group(8)
                        vproj_group(9)
                    elif hp == 0 and kc >= 4 and kc % 2 == 0:
                        vproj_group(9 + (kc - 2) // 2)  # kc=4..14 -> sc=10..15
                    if hp == 0:
                        if kc >= 3 and kc % 2 == 1 and (kc - 3) // 2 < len(ng):
                            proj_group(*ng[(kc - 3) // 2])
                    elif 2 <= kc < 2 + len(ng):
                        proj_group(*ng[kc - 2])
                    for ab in range(2):
                        h = 2 * hp + ab
                        pr0, pr1 = ab * 64, (ab + 1) * 64
                        if (ab, kc // 4) not in pmt:
                            pmt[(ab, kc // 4)] = pm_pool.tile(
                                [P, 4, SQ], BF16, tag="pm", name=f"pm_{ab}_{kc//4}"
                            )
                        pm = pmt[(ab, kc // 4)]
                        sc_ps = psum_sc.tile([P, SQ], F32, tag="scores")
                        for qt in range(QT2):
                            nc.tensor.matmul(
                                sc_ps[:, qt * 512 : (qt + 1) * 512],
                                lhsT=KT_sb[pr0:pr1, hp, kc * P : (kc + 1) * P],
                                rhs=QT_sb[pr0:pr1, hp, qt * 512 : (qt + 1) * 512],
                                start=True,
                                stop=True,
                                tile_position=(pr0, 0),
                            )
                        nc.scalar.activation(pm[:, kc % 4, :], sc_ps[:], EXP, scale=0.125)
                        nc.vector.tensor_tensor(
                            pm[:, kc % 4, :], pm[:, kc % 4, :], maskT_sb[:, kc, :], MUL
                        )
                    for ab in range(2):
                        sh = 3
                        for kcd in ([kc - sh] if kc >= sh else []) + (list(range(kc - sh + 1, kc + 1)) if kc == KC - 1 else []):
                            h = 2 * hp + ab
                            pmd = pmt[(ab, kcd // 4)]
                            for qt in range(QT2):
                                nc.tensor.matmul(
                                    aos[ab][qt][:],
                                    lhsT=V_sb[:, kcd, h, :],
                                    rhs=pmd[:, kcd % 4, qt * 512 : (qt + 1) * 512],
                                    start=(kcd == 0),
                                    stop=(kcd == KC - 1),
                                )
                # normalize: AOT[0:64] * (1/AOT[64]) bcast over partitions;
                # on the last head pair, out-projection per q-tile:
                # hc 0..2 partials start as soon as PSUM slots free (AOT for
                # hp0-2 is long done); only hc3 + bias wait on the normalize.
                fouts = {}

                def fout_partial(qp):
                    fps = psum_sc.tile([P, 2, E], F32, tag="scores", name=f"fout{qp}")
                    fouts[qp] = fps
                    for half in range(2):
                        qc = 2 * qp + half
                        for hc in range(HC - 1):
                            nc.tensor.matmul(
                                fps[:, half, :],
                                lhsT=AOT_sb[:, hc, qc * P : (qc + 1) * P],
                                rhs=wo_bf[:, hc, :],
                                start=(hc == 0),
                                stop=False,
                            )

                def fout_finish(qp):
                    fps = fouts[qp]
                    for half in range(2):
                        qc = 2 * qp + half
                        nc.tensor.matmul(
                            fps[:, half, :],
                            lhsT=AOT_sb[:, HC - 1, qc * P : (qc + 1) * P],
                            rhs=wo_bf[:, HC - 1, :],
                            start=False,
                            stop=False,
                        )
                        nc.tensor.matmul(
                            fps[:, half, :],
                            lhsT=ones_sb[:],
                            rhs=bo_sb[:],
                            start=False,
                            stop=True,
                        )
                    fo = work.tile([P, 2, E], F32, tag="fout", bufs=2)
                    nc.scalar.copy(fo[:], fps[:])
                    nc.sync.dma_start(
                        out_d[2 * qp * P : 2 * (qp + 1) * P, :].rearrange(
                            "(c p) n -> p c n", p=P
                        ),
                        fo[:],
                    )

                if hp == HC - 1:
                    fout_partial(0)
                    fout_partial(1)
                for qt in range(QT2):
                    qs = slice(qt * 512, (qt + 1) * 512)
                    for ab in range(2):
                        pr0, pr1 = ab * 64, (ab + 1) * 64
                        with tc.high_priority(offset=200):
                            rc = work.tile([1, 512], F32, tag="recip")
                            nc.vector.reciprocal(rc[:], aos[ab][qt][64:65, :])
                            rcb = work.tile([64, 512], F32, tag="rcb")
                            nc.gpsimd.partition_broadcast(rcb[:], rc[0:1, :])
                            nc.vector.tensor_tensor(
                                AOT_sb[pr0:pr1, hp, qs],
                                aos[ab][qt][0:64, :],
                                rcb[:],
                                MUL,
                            )
                    if hp == HC - 1:
                        for qp in range(2 * qt, 2 * (qt + 1)):
                            fout_finish(qp)
                        if qt == 0:
                            fout_partial(2)
                            fout_partial(3)

    nc.finalize()
    return nc


_NC_CACHE = {}


def _get_nc(reps: int = 1):
    if reps not in _NC_CACHE:
        _NC_CACHE[reps] = build_nc(reps)
    return _NC_CACHE[reps]


def make_in_maps(input_q, input_k, input_v, mask, wq, bq, wk, bk, wv, bv, wo, bo):
    input_q = np.asarray(input_q, np.float32)
    input_k = np.asarray(input_k, np.float32)
    input_v = np.asarray(input_v, np.float32)
    mask = np.asarray(mask)
    f = np.float32
    h = np.float16
    wq, wk, wv = (np.ascontiguousarray(w).astype(h) for w in (wq, wk, wv))
    wo = np.ascontiguousarray(wo, f)
    bq_pp = np.ascontiguousarray(np.asarray(bq, f).reshape(HC, P).T)
    bk_pp = np.ascontiguousarray(np.asarray(bk, f).reshape(HC, P).T)
    bv_bc = np.ascontiguousarray(np.broadcast_to(np.asarray(bv, f), (P, HD)))
    bo_row = np.ascontiguousarray(np.asarray(bo, f).reshape(1, E)).astype(h)
    kT = [np.ascontiguousarray(input_k[b].T).astype(h) for b in range(B)]
    vT = [np.ascontiguousarray(input_v[b].T).astype(h) for b in range(B)]
    in_maps = []
    for c in range(N_CORES):
        b, qh = c // 2, c % 2
        qs = slice(qh * SQ, (qh + 1) * SQ)
        in_maps.append(
            {
                "xqT": np.ascontiguousarray(input_q[b, qs].T).astype(h),
                "xkT": kT[b],
                "xvT": vT[b],
                "maskT": np.ascontiguousarray(mask[b, qs].T).astype(np.float16),
                "wq": wq,
                "wk": wk,
                "wv": wv,
                "wo_bf": wo.astype(np.float16),
                "bq_pp": bq_pp,
                "bk_pp": bk_pp,
                "bv_bc": bv_bc,
                "bo_row": bo_row,
            }
        )
    return in_maps


def kernel(input_q, input_k, input_v, mask, wq, bq, wk, bk, wv, bv, wo, bo, **_kw):
    nc = _get_nc()
    in_maps = make_in_maps(
        input_q, input_k, input_v, mask, wq, bq, wk, bk, wv, bv, wo, bo
    )
    res = run_bass_kernel_spmd(nc, in_maps, core_ids=list(range(N_CORES)))
    out = np.empty((B, S, E), np.float32)
    for c in range(N_CORES):
        b, qh = c // 2, c % 2
        out[b, qh * SQ : (qh + 1) * SQ] = res.results[c]["out"]
    return out


if __name__ == "__main__":
    rng = np.random.default_rng(0)
    print("building...")
    _get_nc()
    print("built ok")

